# revision 11
# baseline (speedup 1.0000x reference)
"""Trainium2 Bass kernel for nn_Attention_52012053955205.

Multi-head causal attention, B=2 S=2048 D=1024 H=16 HD=64, fp32.

Sharding: 8 cores = 2-way batch x 4-way heads. Each core computes, for its
batch item b and its 4 heads, the partial output sum_h z_h @ W_O_h  as a
full [S, D] tile; the host sums the 4 partials per batch and adds b_O.

Per-core dataflow (everything "transposed" so the softmax denominator is a
free by-product of matmuls):
  xT [D, S] (host-pretransposed) -> QT/KT [d_pair=128, S] via projection
  matmuls (W packed per head-pair, 1/sqrt(HD) folded into W_Q host-side;
  b_Q/b_K added via the ACT-copy per-partition bias).
  V [s, 4*64] natural layout, b_V added via a DVE add with a
  partition-broadcast bias tile; a ones column is appended per head
  (V' [s, 65]) so the z-matmul also produces the softmax denominator.
  Scores TRANSPOSED: S_T[k_tile, q] = KT_tile.T @ QT_block (K=64), causal
  mask added for diagonal tiles (additive -1e9 from a host-built table),
  exp on ScalarE (no max-subtraction: scores are bounded, exp fits fp32),
  z_unnorm^T [65, q] accumulated over k tiles in PSUM (row 64 = denom).
  Normalization: reciprocal of denom row, broadcast to 128 partitions with
  a tiny K=2 matmul against a selector constant, one DVE multiply.
  Output projection: out[s, D] = znorm_pair^T.T @ W_O_pair, accumulated
  over the two head pairs in PSUM; DMA'd out contiguously.
"""

import json

import numpy as np

B, S, D, H, HD = 2, 2048, 1024, 16, 64
NCORES = 8
HPC = 4  # heads per core
MASK_VAL = -1.0e9

_STATE = None


# ---------------------------------------------------------------------------
# Tile tail-drain workaround: walrus in this container rejects >2 sem waits
# on one instruction ("Too many sync wait commands"). Split the tail waits
# across one sync NOP per logical proc; the drain itself then needs none.
# ---------------------------------------------------------------------------
def _patch_tile_drain():
    import concourse.tile as tile
    from concourse.vector_clock import ScopedClock, VectorClock

    if getattr(tile.TileContext, "_drain_split_patch", False):
        return

    def _split_drain_and_barrier(self, tick_clock, wait_clock):
        gc = tick_clock.global_clock
        n = len(gc)
        for proc in range(n):
            t = gc[proc]
            if t > 0:
                vc = VectorClock([t if i == proc else 0 for i in range(n)])
                nop = self.nc.sync.nop(nofuse=True)
                wait_clock.add_sem_waits(nop.ins, ScopedClock({None: vc}))
        self.nc.sync.drain()
        self.nc.all_engine_barrier()
        assert self.sems is not None
        popped = self.nc._tile_sem_poison_stack.pop()
        assert popped is self._sem_poison
        self.nc.clear_and_free_semaphores(list(self.sems.allocated().values()))
        self.nc.all_engine_barrier()

    tile.TileContext._drain_and_barrier = _split_drain_and_barrier
    tile.TileContext._drain_split_patch = True


def _split_waits_bir(bir: bytes) -> bytes:
    """Walrus in this container allows only one sem wait per instruction.
    Spill extra on_wait entries onto same-engine NoOps inserted right
    before the instruction (the NX executes them in stream order)."""
    d = json.loads(bir)
    ctr = 0
    for f in d["functions"]:
        for bb in f["blocks"]:
            new = []
            for ins in bb["instructions"]:
                si = ins.get("sync_info")
                waits = si.get("on_wait", []) if si else []
                if len(waits) > 1:
                    for w in waits[:-1]:
                        ctr += 1
                        new.append(
                            {
                                "debug": ins.get("debug", 0),
                                "engine": ins["engine"],
                                "ins": [],
                                "name": f"I-wsplit-{ctr}",
                                "opcode": "NoOp",
                                "outs": [],
                                "sync_info": {"on_update": [], "on_wait": [w]},
                            }
                        )
                    si["on_wait"] = [waits[-1]]
                new.append(ins)
            bb["instructions"] = new
    return json.dumps(d).encode()


def _hook_wait_split(nc):
    orig = nc.to_json_bytes

    def patched():
        return _split_waits_bir(orig())

    nc.to_json_bytes = patched
    return nc


# ---------------------------------------------------------------------------
# Bass program (identical on all 8 cores; all per-core data arrives as
# ExternalInputs)
# ---------------------------------------------------------------------------
def _build_nc():
    import concourse.bass as bass
    import concourse.mybir as mybir
    import concourse.tile as tile

    FP = mybir.dt.float32
    AF = mybir.ActivationFunctionType
    _patch_tile_drain()

    nc = bass.Bass(target_bir_lowering=False)

    xT = nc.dram_tensor("xt", [D, S], FP, kind="ExternalInput")
    wq = nc.dram_tensor("wq", [2, D, 128], FP, kind="ExternalInput")
    wk = nc.dram_tensor("wk", [2, D, 128], FP, kind="ExternalInput")
    wv = nc.dram_tensor("wv", [D, 256], FP, kind="ExternalInput")
    wo = nc.dram_tensor("wo", [2, 128, D], FP, kind="ExternalInput")
    bq = nc.dram_tensor("bq", [2, 128], FP, kind="ExternalInput")
    bk = nc.dram_tensor("bk", [2, 128], FP, kind="ExternalInput")
    bv = nc.dram_tensor("bv", [256], FP, kind="ExternalInput")
    masks = nc.dram_tensor("masks", [4, 128, 512], FP, kind="ExternalInput")
    sel = nc.dram_tensor("sel", [2, 128], FP, kind="ExternalInput")
    out = nc.dram_tensor("out", [S, D], FP, kind="ExternalOutput")

    with tile.TileContext(nc) as tc:
        with (
            tc.tile_pool(name="consts", bufs=1) as consts,
            tc.tile_pool(name="xp", bufs=2) as xp,
            tc.tile_pool(name="qk", bufs=1) as qk,
            tc.tile_pool(name="vp", bufs=1) as vp,
            tc.tile_pool(name="zp", bufs=1) as zp,
            tc.tile_pool(name="etp", bufs=4) as etp,
            tc.tile_pool(name="bcp", bufs=2) as bcp,
            tc.tile_pool(name="rdpool", bufs=4) as rdpool,
            tc.tile_pool(name="ostp", bufs=3) as ostp,
            tc.tile_pool(name="psA", bufs=3, space="PSUM") as psA,
            tc.tile_pool(name="psZ", bufs=3, space="PSUM") as psZp,
            tc.tile_pool(name="psB", bufs=2, space="PSUM") as psB,
        ):
            # ---- constants ----
            wq_sb = consts.tile([128, 2, 8, 128], FP, tag="wq")
            nc.sync.dma_start(
                out=wq_sb, in_=wq[:].rearrange("a (c p) d -> p a c d", p=128)
            )
            wk_sb = consts.tile([128, 2, 8, 128], FP, tag="wk")
            nc.sync.dma_start(
                out=wk_sb, in_=wk[:].rearrange("a (c p) d -> p a c d", p=128)
            )
            wv_sb = consts.tile([128, 8, 256], FP, tag="wv")
            nc.sync.dma_start(
                out=wv_sb, in_=wv[:].rearrange("(c p) d -> p c d", p=128)
            )
            wo_sb = consts.tile([128, 2, D], FP, tag="wo")
            nc.sync.dma_start(out=wo_sb, in_=wo[:].rearrange("a p d -> p a d"))
            masks_sb = consts.tile([128, 4, 512], FP, tag="masks")
            nc.sync.dma_start(
                out=masks_sb, in_=masks[:].rearrange("m p j -> p m j")
            )
            sel_sb = consts.tile([1, 2, 128], FP, tag="sel")
            import concourse.bass as _b0

            nc.sync.dma_start(
                out=sel_sb,
                in_=_b0.AP(tensor=sel, offset=0, ap=[[256, 1], [128, 2], [1, 128]]),
            )
            bq_sb = consts.tile([128, 2], FP, tag="bq")
            nc.sync.dma_start(out=bq_sb, in_=bq[:].rearrange("a p -> p a"))
            bk_sb = consts.tile([128, 2], FP, tag="bk")
            nc.sync.dma_start(out=bk_sb, in_=bk[:].rearrange("a p -> p a"))
            bvbc_sb = consts.tile([128, 4, 64], FP, tag="bvbc")
            import concourse.bass as _b

            nc.sync.dma_start(
                out=bvbc_sb,
                in_=_b.AP(tensor=bv, offset=0, ap=[[0, 128], [1, 256]]),
            )

            qt_sb = qk.tile([128, 2, S], FP, tag="qt")
            kt_sb = qk.tile([128, 2, S], FP, tag="kt")
            v_sb = vp.tile([128, 16, 4, 65], FP, tag="v")
            znp = zp.tile([128, 2, 4, 512], FP, tag="zn")

            # ones column of V' (written once; phase-1 fills the rest)
            nc.vector.memset(v_sb[:, :, :, 64:65], 1.0)

            # ---- phase 1: projections, per s-block of 512 ----
            xTr = xT[:].rearrange("(c p) s -> p c s", p=128)
            for sb in range(4):
                x_t = xp.tile([128, 8, 512], FP, tag="x")
                nc.sync.dma_start(
                    out=x_t, in_=xTr[:, :, sb * 512 : (sb + 1) * 512]
                )
                for pair in range(2):
                    psQ = psA.tile([128, 512], FP, tag="A")
                    for c in range(8):
                        nc.tensor.matmul(
                            psQ,
                            wq_sb[:, pair, c, :],
                            x_t[:, c, :],
                            start=(c == 0),
                            stop=(c == 7),
                        )
                    nc.scalar.activation(
                        qt_sb[:, pair, sb * 512 : (sb + 1) * 512],
                        psQ,
                        AF.Identity,
                        bias=bq_sb[:, pair : pair + 1],
                    )
                    psK = psA.tile([128, 512], FP, tag="A")
                    for c in range(8):
                        nc.tensor.matmul(
                            psK,
                            wk_sb[:, pair, c, :],
                            x_t[:, c, :],
                            start=(c == 0),
                            stop=(c == 7),
                        )
                    nc.scalar.activation(
                        kt_sb[:, pair, sb * 512 : (sb + 1) * 512],
                        psK,
                        AF.Identity,
                        bias=bk_sb[:, pair : pair + 1],
                    )
                for stl in range(4):
                    st = sb * 4 + stl
                    psV = psA.tile([128, 256], FP, tag="A")
                    for c in range(8):
                        nc.tensor.matmul(
                            psV,
                            x_t[:, c, stl * 128 : (stl + 1) * 128],
                            wv_sb[:, c, :],
                            start=(c == 0),
                            stop=(c == 7),
                        )
                    nc.vector.tensor_add(
                        v_sb[:, st, :, 0:64],
                        psV.rearrange("p (h d) -> p h d", h=4),
                        bvbc_sb,
                    )

            # ---- phase 2: attention per head pair / q-block ----
            for pair in range(2):
                for Qb in range(4):
                    q0, q1 = Qb * 512, (Qb + 1) * 512
                    ktmax = 4 * (Qb + 1)
                    rds = []
                    psZs = []
                    for hh in range(2):
                        po = hh * 64
                        psZ = psZp.tile([65, 512], FP, tag="Z")
                        psZs.append(psZ)
                        for kt in range(ktmax):
                            psS = psA.tile([128, 512], FP, tag="A")
                            nc.tensor.matmul(
                                psS,
                                kt_sb[po : po + 64, pair, kt * 128 : (kt + 1) * 128],
                                qt_sb[po : po + 64, pair, q0:q1],
                                start=True,
                                stop=True,
                            )
                            if kt >= 4 * Qb:
                                nc.vector.tensor_add(
                                    psS, psS, masks_sb[:, kt % 4, :]
                                )
                            e_t = etp.tile([128, 512], FP, tag="et")
                            nc.scalar.activation(e_t, psS, AF.Exp)
                            nc.tensor.matmul(
                                psZ,
                                v_sb[:, kt, 2 * pair + hh, :],
                                e_t,
                                start=(kt == 0),
                                stop=(kt == ktmax - 1),
                            )
                        rd_h = rdpool.tile([1, 512], FP, tag="rd")
                        rds.append(rd_h)
                        nc.vector.reciprocal(rd_h, psZ[64:65, :])
                    # broadcast 1/denom of both heads to a stacked [128, 512]
                    # tile via two K=1 matmuls against selector rows
                    bc = psB.tile([128, 512], FP, tag="B")
                    nc.tensor.matmul(
                        bc, sel_sb[:, 0, :], rds[0], start=True, stop=False
                    )
                    nc.tensor.matmul(
                        bc, sel_sb[:, 1, :], rds[1], start=False, stop=True
                    )
                    bcs = bcp.tile([128, 512], FP, tag="bcs")
                    nc.scalar.activation(bcs, bc, AF.Copy)
                    # hh=0: partitions already 0..63 everywhere
                    nc.vector.tensor_mul(
                        znp[0:64, pair, Qb, :],
                        psZs[0][0:64, :],
                        bcs[0:64, :],
                    )
                    # hh=1: single-src shift-copy 0..63 -> 64..127, then mul
                    zc = bcp.tile([128, 512], FP, tag="zc")
                    nc.vector.tensor_copy(zc[64:128, :], psZs[1][0:64, :])
                    nc.vector.tensor_mul(
                        znp[64:128, pair, Qb, :],
                        zc[64:128, :],
                        bcs[64:128, :],
                    )

            # ---- phase 3: output projection ----
            for st in range(16):
                Qb, soff = st // 4, (st % 4) * 128
                ost_t = ostp.tile([128, D], FP, tag="ost")
                for Db in range(2):
                    psO = psB.tile([128, 512], FP, tag="B")
                    for pair in range(2):
                        nc.tensor.matmul(
                            psO,
                            znp[:, pair, Qb, soff : soff + 128],
                            wo_sb[:, pair, Db * 512 : (Db + 1) * 512],
                            start=(pair == 0),
                            stop=(pair == 1),
                        )
                    nc.vector.tensor_copy(
                        ost_t[:, Db * 512 : (Db + 1) * 512], psO
                    )
                nc.sync.dma_start(
                    out=out[st * 128 : (st + 1) * 128, :], in_=ost_t
                )

    return _hook_wait_split(nc)


# ---------------------------------------------------------------------------
# Persistent PJRT runner (mirrors run_bass_via_pjrt, but keeps the jitted
# callable so repeated kernel() calls don't recompile)
# ---------------------------------------------------------------------------
class _Runner:
    def __init__(self, nc):
        import jax
        import jax.numpy as jnp  # noqa: F401
        import numpy as _np
        from jax.experimental.shard_map import shard_map
        from jax.sharding import Mesh, PartitionSpec
        import concourse.mybir as mybir
        from concourse.bass2jax import (
            _bass_exec_p,
            install_neuronx_cc_hook,
            partition_id_tensor,
        )

        install_neuronx_cc_hook()
        self.jax = jax
        pname = nc.partition_id_tensor.name if nc.partition_id_tensor else None
        in_names, out_names, out_avals, zero_outs = [], [], [], []
        for alloc in nc.m.functions[0].allocations:
            if not isinstance(alloc, mybir.MemoryLocationSet):
                continue
            name = alloc.memorylocations[0].name
            if alloc.kind == "ExternalInput":
                if name == pname:
                    continue
                in_names.append(name)
            elif alloc.kind == "ExternalOutput":
                shape = tuple(alloc.tensor_shape)
                dtype = mybir.dt.np(alloc.dtype)
                out_names.append(name)
                out_avals.append(jax.core.ShapedArray(shape, dtype))
                zero_outs.append(_np.zeros(shape, dtype))
        self.in_names, self.out_names = list(in_names), list(out_names)
        self.out_avals, self.zero_outs = out_avals, zero_outs
        n_params, n_outs = len(in_names), len(out_names)
        self.n_params = n_params
        all_names = in_names + out_names
        if pname is not None:
            all_names = all_names + [pname]

        def _body(*args):
            operands = list(args)
            if pname is not None:
                operands.append(partition_id_tensor())
            outs = _bass_exec_p.bind(
                *operands,
                out_avals=tuple(out_avals),
                in_names=tuple(all_names),
                out_names=tuple(out_names),
                lowering_input_output_aliases=(),
                sim_require_finite=True,
                sim_require_nnan=True,
                nc=nc,
            )
            return tuple(outs)

        devices = jax.devices()[:NCORES]
        mesh = Mesh(np.asarray(devices), ("core",))
        in_specs = (PartitionSpec("core"),) * (n_params + n_outs)
        out_specs = (PartitionSpec("core"),) * n_outs
        self.fn = jax.jit(
            shard_map(
                _body,
                mesh=mesh,
                in_specs=in_specs,
                out_specs=out_specs,
                check_rep=False,
            ),
            donate_argnums=tuple(range(n_params, n_params + n_outs)),
            keep_unused=True,
        )

    def concat_inputs(self, in_maps):
        return [
            np.concatenate([in_maps[c][n] for c in range(NCORES)], axis=0)
            for n in self.in_names
        ]

    def run_concat(self, concat_in):
        zeros = [
            np.zeros((NCORES * z.shape[0], *z.shape[1:]), z.dtype)
            for z in self.zero_outs
        ]
        outs = self.fn(*concat_in, *zeros)
        outs = [np.asarray(o) for o in outs]
        return outs

    def run(self, in_maps):
        outs = self.run_concat(self.concat_inputs(in_maps))
        per_core = []
        for c in range(NCORES):
            m = {}
            for i, n in enumerate(self.out_names):
                shp = self.out_avals[i].shape
                m[n] = outs[i].reshape(NCORES, *shp)[c]
            per_core.append(m)
        return per_core


def _make_masks():
    m = np.zeros((4, 128, 512), dtype=np.float32)
    for r in range(4):
        p = np.arange(128)[:, None]
        j = np.arange(512)[None, :]
        m[r][p + 128 * r > j] = MASK_VAL
    return m


def _prep_core_inputs(inputs):
    """Shard + repack the full problem inputs into per-core input maps."""
    x = np.asarray(inputs["normalized_resid_pre"], dtype=np.float32)
    W_Q = np.asarray(inputs["W_Q"], dtype=np.float32)
    W_K = np.asarray(inputs["W_K"], dtype=np.float32)
    W_V = np.asarray(inputs["W_V"], dtype=np.float32)
    W_O = np.asarray(inputs["W_O"], dtype=np.float32)
    b_Q = np.asarray(inputs["b_Q"], dtype=np.float32)
    b_K = np.asarray(inputs["b_K"], dtype=np.float32)
    b_V = np.asarray(inputs["b_V"], dtype=np.float32)

    scale = np.float32(1.0 / np.sqrt(HD))
    masks = _make_masks()
    sel = np.zeros((2, 128), dtype=np.float32)
    sel[0, 0:64] = 1.0
    sel[1, 64:128] = 1.0

    in_maps = []
    for c in range(NCORES):
        b, g = c // 4, c % 4
        hs = [4 * g + i for i in range(HPC)]
        xTb = np.ascontiguousarray(x[b].T)  # [D, S]
        wq_p = np.zeros((2, D, 128), dtype=np.float32)
        wk_p = np.zeros((2, D, 128), dtype=np.float32)
        wo_p = np.zeros((2, 128, D), dtype=np.float32)
        bq_p = np.zeros((2, 128), dtype=np.float32)
        bk_p = np.zeros((2, 128), dtype=np.float32)
        for pr in range(2):
            h0, h1 = hs[2 * pr], hs[2 * pr + 1]
            wq_p[pr, :, 0:64] = W_Q[h0] * scale
            wq_p[pr, :, 64:128] = W_Q[h1] * scale
            wk_p[pr, :, 0:64] = W_K[h0]
            wk_p[pr, :, 64:128] = W_K[h1]
            wo_p[pr, 0:64, :] = W_O[h0]
            wo_p[pr, 64:128, :] = W_O[h1]
            bq_p[pr, 0:64] = b_Q[h0] * scale
            bq_p[pr, 64:128] = b_Q[h1] * scale
            bk_p[pr, 0:64] = b_K[h0]
            bk_p[pr, 64:128] = b_K[h1]
        wv_p = np.concatenate([W_V[h] for h in hs], axis=1)  # [D, 256]
        bv_p = np.concatenate([b_V[h] for h in hs], axis=0)  # [256]
        in_maps.append(
            {
                "xt": xTb,
                "wq": wq_p,
                "wk": wk_p,
                "wv": np.ascontiguousarray(wv_p),
                "wo": wo_p,
                "bq": bq_p,
                "bk": bk_p,
                "bv": np.ascontiguousarray(bv_p),
                "masks": masks,
                "sel": sel,
            }
        )
    return in_maps


def _get_state():
    global _STATE
    if _STATE is None:
        nc = _build_nc()
        _STATE = _Runner(nc)
    return _STATE


def kernel(**inputs):
    st = _get_state()
    in_maps = _prep_core_inputs(inputs)
    per_core = st.run(in_maps)
    b_O = np.asarray(inputs["b_O"], dtype=np.float32)
    out = np.zeros((B, S, D), dtype=np.float32)
    for c in range(NCORES):
        out[c // 4] += per_core[c]["out"]
    out += b_O[None, None, :]
    return out
